# revision 1
# baseline (speedup 1.0000x reference)
"""Trainium2 Bass kernel for nn_DenSparseMatrix (segment_reduce).

out[b, o] = sum_{i,m: mapping[i,m]==o} weights[i,m] * x[b,i]
  x: (32, 65536) f32, weights/mapping: (65536, 32), output (32, 65536) f32.

Strategy (8 NeuronCores, full-input contract):
  * Host sorts the 2M (i,m) entries by output index o; cores are sharded by
    o-range (8192 outputs each) so no cross-core reduction is needed.
  * Each o's entry list is cut into fixed-size chunks (32/16/8 slots, zero
    padded).  Chunk slots are mapped onto SBUF partitions; each
    indirect-DMA call gathers 128 x-rows (one per partition) from the
    x^T table in HBM -- the only data-dependent addressing this stack
    supports.
  * DVE multiplies gathered rows by their weights; the TensorEngine
    contracts each 128-partition group with a constant block-ones matrix,
    producing per-chunk partial sums in PSUM.
  * Host adds per-chunk partials into the final (B, O) output (cheap
    O(#chunks) assembly of device-computed sums).
"""

import numpy as np

# ---------------------------------------------------------------------------
# Tile framework compatibility patches (this walrus build allows only one
# sync-wait per instruction; TileContext can attach more).
# ---------------------------------------------------------------------------


def _apply_tile_patches():
    import concourse.tile as tile_mod
    from concourse import mybir
    from concourse.vector_clock import ScopedClock

    def _split_drain_and_barrier(self, tick_clock, wait_clock):
        nc = self.nc
        drain_inst = nc.sync.drain()
        wait_clock.add_sem_waits(
            drain_inst.ins, ScopedClock({None: tick_clock.global_clock})
        )
        si = drain_inst.ins.sync_info
        if si is not None and len(si.on_wait) > 1:
            waits = list(si.on_wait)
            si.on_wait.clear()
            si.on_wait.append(waits[0])
            for w in waits[1:]:
                extra = nc.sync.drain()
                esi = extra.ins.sync_info
                if esi is None:
                    extra.ins.sync_info = mybir.SyncInfo(
                        on_wait=[w], on_update=[]
                    )
                else:
                    esi.on_wait.append(w)
        nc.all_engine_barrier()
        assert self.sems is not None
        popped = nc._tile_sem_poison_stack.pop()
        assert popped is self._sem_poison
        nc.clear_and_free_semaphores(list(self.sems.allocated().values()))
        nc.all_engine_barrier()

    tile_mod.TileContext._drain_and_barrier = _split_drain_and_barrier


def _legalize_waits(nc):
    from concourse import mybir

    for bb_name, bass_bb in nc.bb_map.items():
        bb = bass_bb.bb
        insts = bb.instructions
        out = []
        changed = False
        for inst in insts:
            si = inst.sync_info
            if si is not None and si.on_wait is not None and len(si.on_wait) > 1:
                waits = list(si.on_wait)
                si.on_wait.clear()
                si.on_wait.append(waits[0])
                eng = nc.engines[inst.engine]
                for w in waits[1:]:
                    nop = eng.nop(nofuse=True, hint="wait_split")
                    cur_list = nc.cur_bb.bb.instructions
                    assert cur_list and cur_list[-1].name == nop.ins.name
                    cur_list.pop()
                    if nop.ins.sync_info is None:
                        nop.ins.sync_info = mybir.SyncInfo(
                            on_wait=[w], on_update=[]
                        )
                    else:
                        nop.ins.sync_info.on_wait.append(w)
                    out.append(nop.ins)
                changed = True
            out.append(inst)
        if changed:
            insts[:] = out


# ---------------------------------------------------------------------------
# Problem constants
# ---------------------------------------------------------------------------
B = 32          # batch
I = 65536       # input size
M = 32          # mapping width
O = 65536       # output size
N_CORES = 8
O_PER_CORE = O // N_CORES      # 8192
P = 128                        # SBUF partitions
GROUP = 16                     # indirect calls per matmul group (16*32=512)
CHUNK_SIZES = (32, 16, 8, 4)   # slot-chunk granularities (binary tail)

_CACHE = {}


def _plan_core(entry_i, entry_w, counts):
    """Build per-chunk-size (idx, w, chunk_o) arrays for one core.

    Each output row's count c is decomposed as 32-chunks plus a tail of
    {16, 8, 4}-chunks with at most 3 padded slots:
      c = 32*n32 + r;  r = 16*t16 + 8*t8a + tail;  tail<=4 -> one 4-chunk,
      tail in 5..7 -> one 8-chunk.
    Returns dict L -> [idx_2d (n_chunks, L), w_2d, chunk_o (n_chunks,)].
    """
    n_o = counts.shape[0]
    starts = np.zeros(n_o + 1, dtype=np.int64)
    np.cumsum(counts, out=starts[1:])
    n32 = counts // 32
    r = counts - 32 * n32
    t16 = (r >= 16).astype(np.int64)
    r2 = r - 16 * t16
    t8a = (r2 >= 8).astype(np.int64)
    r3 = r2 - 8 * t8a
    t8b = (r3 >= 5).astype(np.int64)
    t4 = ((r3 >= 1) & (r3 <= 4)).astype(np.int64)

    n_per = {32: n32, 16: t16, 8: t8a + t8b, 4: t4}

    ranks = np.arange(entry_i.shape[0], dtype=np.int64)
    o_of_entry = np.repeat(np.arange(n_o, dtype=np.int64), counts)
    q = ranks - starts[o_of_entry]           # rank within o

    b0 = 32 * n32
    b1 = b0 + 16 * t16
    b2 = b1 + 8 * t8a

    out = {}
    for L in (32, 16, 8, 4):
        nL = n_per[L]
        n_chunks = int(nL.sum())
        chunk_start = np.zeros(n_o + 1, dtype=np.int64)
        np.cumsum(nL, out=chunk_start[1:])
        oe = o_of_entry
        if L == 32:
            sel = q < b0[oe]
            local = q[sel] - 0
            row = chunk_start[oe[sel]] + local // 32
            col = local % 32
        elif L == 16:
            sel = (q >= b0[oe]) & (q < b1[oe])
            row = chunk_start[oe[sel]]
            col = q[sel] - b0[oe[sel]]
        elif L == 8:
            sel_a = (q >= b1[oe]) & (q < b2[oe])
            sel_b = (q >= b2[oe]) & (t8b[oe] == 1)
            sel = sel_a | sel_b
            row = np.where(
                sel_a[sel] if False else (q[sel] < b2[oe[sel]]),
                chunk_start[oe[sel]],
                chunk_start[oe[sel]] + t8a[oe[sel]],
            )
            col = np.where(
                q[sel] < b2[oe[sel]],
                q[sel] - b1[oe[sel]],
                q[sel] - b2[oe[sel]],
            )
        else:  # L == 4
            sel = (q >= b2[oe]) & (t4[oe] == 1)
            row = chunk_start[oe[sel]]
            col = q[sel] - b2[oe[sel]]
        idx2 = np.zeros((n_chunks, L), dtype=np.int64)
        w2 = np.zeros((n_chunks, L), dtype=np.float32)
        idx2[row, col] = entry_i[sel]
        w2[row, col] = entry_w[sel]
        chunk_o = np.repeat(np.arange(n_o, dtype=np.int64), nL)
        out[L] = [idx2, w2, chunk_o]
    return out


def _pack_calls(idx2, w2, L, n_calls):
    """Pack (n_chunks, L) chunk arrays into call matrices (P, n_calls).

    Call r holds chunks [r*(P//L), (r+1)*(P//L)); chunk j -> partitions
    [j*L, (j+1)*L).  Returns idx (P, n_calls) int32, w (P, n_calls) f32.
    """
    g = P // L
    n_chunks = idx2.shape[0]
    idx_full = np.zeros((n_calls * g, L), dtype=np.int64)
    w_full = np.zeros((n_calls * g, L), dtype=np.float32)
    idx_full[:n_chunks] = idx2
    w_full[:n_chunks] = w2
    # call r, partition p = j*L+s  ->  chunk r*g + j, slot s
    idxm = idx_full.reshape(n_calls, g * L).T.astype(np.int32)
    wm = w_full.reshape(n_calls, g * L).T
    return np.ascontiguousarray(idxm), np.ascontiguousarray(wm)


def _build_program(n_calls_by_L, repeat=1, bufs=(12, 8, 4)):
    """Construct the Bass SPMD program. n_calls_by_L: {L: n_calls} (uniform
    across cores; multiples of GROUP). repeat>1 re-traces the whole body
    (timing amplification; outputs are simply overwritten)."""
    import concourse.bass as bass
    import concourse.mybir as mybir
    from concourse.bass import IndirectOffsetOnAxis
    from concourse import tile

    _apply_tile_patches()

    nc = bass.Bass(num_swdge_queues=2)
    table = nc.declare_dram_parameter(
        "xT", [I, B], mybir.dt.float32, isOutput=False
    )
    idx_p = {}
    w_p = {}
    out_p = {}
    for L, n_calls in n_calls_by_L.items():
        if n_calls == 0:
            continue
        idx_p[L] = nc.declare_dram_parameter(
            f"idx{L}", [P, n_calls], mybir.dt.int32, isOutput=False
        )
        w_p[L] = nc.declare_dram_parameter(
            f"w{L}", [P, n_calls], mybir.dt.float32, isOutput=False
        )
        n_groups = n_calls // GROUP
        n_batches = -(-n_groups // 8)
        out_p[L] = nc.declare_dram_parameter(
            f"out{L}", [P // L, n_batches * 4096], mybir.dt.float32,
            isOutput=True,
        )

    with tile.TileContext(nc) as tc:
        with (
            tc.tile_pool(name="meta", bufs=1) as meta_pool,
            tc.tile_pool(name="gath", bufs=bufs[0]) as gath_pool,
            tc.tile_pool(name="psum", bufs=bufs[1], space="PSUM") as psum_pool,
            tc.tile_pool(name="outs", bufs=bufs[2]) as out_pool,
        ):
            ones_t = {}
            idx_t = {}
            w_t = {}
            out_t = {}
            for L, n_calls in n_calls_by_L.items():
                if n_calls == 0:
                    continue
                nb = P // L
                ones = meta_pool.tile([P, nb], mybir.dt.float32, tag=f"ones{L}")
                # block-ones: ones[p, j] = 1 iff p // L == j
                nc.gpsimd.memset(ones[:], 1.0)
                nc.gpsimd.affine_select(
                    out=ones[:], in_=ones[:],
                    compare_op=mybir.AluOpType.is_ge, fill=0.0,
                    base=0, pattern=[[-L, nb]], channel_multiplier=1,
                )
                nc.gpsimd.affine_select(
                    out=ones[:], in_=ones[:],
                    compare_op=mybir.AluOpType.is_ge, fill=0.0,
                    base=L - 1, pattern=[[L, nb]], channel_multiplier=-1,
                )
                ones_t[L] = ones
                it = meta_pool.tile([P, n_calls], mybir.dt.int32, tag=f"idx{L}")
                wt = meta_pool.tile([P, n_calls], mybir.dt.float32, tag=f"w{L}")
                nc.sync.dma_start(out=it[:], in_=idx_p[L][:])
                nc.sync.dma_start(out=wt[:], in_=w_p[L][:])
                idx_t[L] = it
                w_t[L] = wt

            for _rep in range(repeat):
              for L, n_calls in n_calls_by_L.items():
                if n_calls == 0:
                    continue
                nb = P // L
                n_groups = n_calls // GROUP
                batch_t = None
                for g in range(n_groups):
                    gt = gath_pool.tile(
                        [P, GROUP, B], mybir.dt.float32, tag="g"
                    )
                    for c in range(GROUP):
                        r = g * GROUP + c
                        _gi = nc.gpsimd.indirect_dma_start(
                            out=gt[:, c, :],
                            out_offset=None,
                            in_=table[:],
                            in_offset=IndirectOffsetOnAxis(
                                ap=idx_t[L][:, r:r + 1], axis=0
                            ),
                        )
                        if r % 2:
                            _gi.ins.queue = "qPoolDynamic1" 
                    # multiply by weights (broadcast along B)
                    nc.vector.tensor_tensor(
                        out=gt[:],
                        in0=gt[:],
                        in1=w_t[L][:, g * GROUP:(g + 1) * GROUP]
                        .unsqueeze(2).broadcast_to([P, GROUP, B]),
                        op=mybir.AluOpType.mult,
                    )
                    # contract partition blocks of L with block-ones
                    ps = psum_pool.tile(
                        [nb, 512], mybir.dt.float32, tag="ps", space="PSUM"
                    )
                    nc.tensor.matmul(
                        out=ps[:],
                        lhsT=ones_t[L][:],
                        rhs=gt[:].rearrange("p c e -> p (c e)"),
                        start=True,
                        stop=True,
                    )
                    # pack 8 groups per (nb, 4096) batch tile, then DMA out
                    k = g % 8
                    if k == 0:
                        batch_t = out_pool.tile(
                            [nb, 8, 512], mybir.dt.float32, tag="ob"
                        )
                    nc.any.tensor_copy(out=batch_t[:, k, :], in_=ps[:])
                    if k == 7 or g == n_groups - 1:
                        t = g // 8
                        nc.sync.dma_start(
                            out=out_p[L][:, t * 4096:(t + 1) * 4096],
                            in_=batch_t[:].rearrange("p k e -> p (k e)"),
                        )

    _legalize_waits(nc)
    return nc


def _prepare(x, forward_weights, input_mapping):
    """Host-side planning: returns (in_maps, assembly_meta, n_calls_by_L)."""
    xT = np.ascontiguousarray(x.T).astype(np.float32)          # (I, B)
    o_all = np.asarray(input_mapping).reshape(-1).astype(np.int64)
    w_all = np.asarray(forward_weights).reshape(-1).astype(np.float32)
    i_all = np.arange(o_all.shape[0], dtype=np.int64) >> 5

    order = np.argsort(o_all, kind="stable")
    o_s = o_all[order]
    i_s = i_all[order]
    w_s = w_all[order]
    counts_full = np.bincount(o_all, minlength=O)

    core_plans = []
    pos = 0
    for c in range(N_CORES):
        o0 = c * O_PER_CORE
        counts = counts_full[o0:o0 + O_PER_CORE]
        n_ent = int(counts.sum())
        plan = _plan_core(
            i_s[pos:pos + n_ent], w_s[pos:pos + n_ent], counts
        )
        core_plans.append(plan)
        pos += n_ent

    n_calls_by_L = {}
    for L in CHUNK_SIZES:
        g = P // L
        max_chunks = max(p[L][0].shape[0] for p in core_plans)
        n_calls = -(-max_chunks // g)
        n_calls = -(-n_calls // GROUP) * GROUP   # multiple of GROUP
        n_calls_by_L[L] = n_calls

    in_maps = []
    metas = []
    for c in range(N_CORES):
        m = {"xT": xT}
        meta = {}
        for L in CHUNK_SIZES:
            idx2, w2, chunk_o = core_plans[c][L]
            n_calls = n_calls_by_L[L]
            if n_calls == 0:
                continue
            im, wm = _pack_calls(idx2, w2, L, n_calls)
            m[f"idx{L}"] = im
            m[f"w{L}"] = wm
            meta[L] = (chunk_o, idx2.shape[0])
        in_maps.append(m)
        metas.append(meta)
    return in_maps, metas, n_calls_by_L


def _assemble(results, metas, n_calls_by_L):
    """Decode device outputs and accumulate per-chunk sums into (B, O)."""
    out = np.zeros((O, B), dtype=np.float32)
    for c in range(N_CORES):
        o0 = c * O_PER_CORE
        for L in CHUNK_SIZES:
            n_calls = n_calls_by_L[L]
            if n_calls == 0:
                continue
            chunk_o, n_chunks = metas[c][L]
            if n_chunks == 0:
                continue
            nb = P // L
            n_groups = n_calls // GROUP
            n_batches = -(-n_groups // 8)
            raw = results[c][f"out{L}"]           # (nb, n_batches*4096)
            # value at [j, t*4096 + k*512 + cc*32 + b],
            # chunk id = ((t*8+k)*GROUP + cc)*nb + j
            raw = raw.reshape(nb, n_batches, 8, GROUP, B)
            sums = np.transpose(raw, (1, 2, 3, 0, 4)).reshape(
                n_batches * 8 * GROUP * nb, B
            )
            np.add.at(out, chunk_o[:n_chunks] + o0, sums[:n_chunks])
    return np.ascontiguousarray(out.T)


def kernel(x, forward_weights, input_mapping, output_size):
    from concourse.bass_utils import run_bass_kernel_spmd

    x = np.asarray(x)
    forward_weights = np.asarray(forward_weights)
    input_mapping = np.asarray(input_mapping)
    assert int(output_size) == O

    in_maps, metas, n_calls_by_L = _prepare(x, forward_weights, input_mapping)

    key = tuple(sorted(n_calls_by_L.items()))
    if key not in _CACHE:
        _CACHE[key] = _build_program(n_calls_by_L)
    nc = _CACHE[key]

    res = run_bass_kernel_spmd(
        nc, in_maps, core_ids=list(range(N_CORES))
    )
    return _assemble(res.results, metas, n_calls_by_L)



# revision 9
# speedup vs baseline: 2.8879x; 2.8879x over previous
"""Trainium2 Bass kernel for nn_DenSparseMatrix (segment_reduce).

out[b, o] = sum_{i,m: mapping[i,m]==o} weights[i,m] * x[b,i]
  x: (32, 65536) f32, weights/mapping: (65536, 32), output (32, 65536) f32.

Strategy (8 NeuronCores, full-input contract):
  * Host sorts the 2M (i,m) entries by output index o; cores are sharded by
    o-range (8192 outputs each) so no cross-core reduction is needed.
  * Each o's entry list is cut into fixed-size chunks (32/16/8/4 slots,
    zero padded).  Chunk slots are mapped onto (partition, column) cells of
    a flat slot stream; one bulk dma_gather instruction fetches thousands
    of x-rows at once (994ns fixed cost amortized vs. per-128-row indirect
    DMA), reading 256B *pair rows* of the xT table so the pair index fits
    the gather's int16 index type.  Host-prepared weights zero the unused
    half of each pair.
  * DVE multiplies gathered pairs by weights; the TensorEngine contracts
    each 128-partition group with a constant block-ones matrix into PSUM;
    the Activation engine folds the two pair-halves together into SBUF.
  * Host adds per-chunk partials into the final (B, O) output (cheap
    O(#chunks) assembly of device-computed sums).
"""

import numpy as np

# ---------------------------------------------------------------------------
# Tile framework compatibility patches (this walrus build allows only one
# sync-wait per instruction; TileContext can attach more).
# ---------------------------------------------------------------------------


def _apply_tile_patches():
    import concourse.tile as tile_mod
    from concourse import mybir
    from concourse.vector_clock import ScopedClock

    def _split_drain_and_barrier(self, tick_clock, wait_clock):
        nc = self.nc
        drain_inst = nc.sync.drain()
        wait_clock.add_sem_waits(
            drain_inst.ins, ScopedClock({None: tick_clock.global_clock})
        )
        si = drain_inst.ins.sync_info
        if si is not None and len(si.on_wait) > 1:
            waits = list(si.on_wait)
            si.on_wait.clear()
            si.on_wait.append(waits[0])
            for w in waits[1:]:
                extra = nc.sync.drain()
                esi = extra.ins.sync_info
                if esi is None:
                    extra.ins.sync_info = mybir.SyncInfo(
                        on_wait=[w], on_update=[]
                    )
                else:
                    esi.on_wait.append(w)
        nc.all_engine_barrier()
        assert self.sems is not None
        popped = nc._tile_sem_poison_stack.pop()
        assert popped is self._sem_poison
        nc.clear_and_free_semaphores(list(self.sems.allocated().values()))
        nc.all_engine_barrier()

    tile_mod.TileContext._drain_and_barrier = _split_drain_and_barrier


def _legalize_waits(nc):
    from concourse import mybir

    for bb_name, bass_bb in nc.bb_map.items():
        bb = bass_bb.bb
        insts = bb.instructions
        out = []
        changed = False
        for inst in insts:
            si = inst.sync_info
            if si is not None and si.on_wait is not None and len(si.on_wait) > 1:
                waits = list(si.on_wait)
                si.on_wait.clear()
                si.on_wait.append(waits[0])
                eng = nc.engines[inst.engine]
                for w in waits[1:]:
                    nop = eng.nop(nofuse=True, hint="wait_split")
                    cur_list = nc.cur_bb.bb.instructions
                    assert cur_list and cur_list[-1].name == nop.ins.name
                    cur_list.pop()
                    if nop.ins.sync_info is None:
                        nop.ins.sync_info = mybir.SyncInfo(
                            on_wait=[w], on_update=[]
                        )
                    else:
                        nop.ins.sync_info.on_wait.append(w)
                    out.append(nop.ins)
                changed = True
            out.append(inst)
        if changed:
            insts[:] = out


# ---------------------------------------------------------------------------
# Problem constants
# ---------------------------------------------------------------------------
B = 32          # batch
I = 65536       # input size
M = 32          # mapping width
O = 65536       # output size
N_CORES = 8
O_PER_CORE = O // N_CORES      # 8192
P = 128                        # SBUF partitions
CHUNK_SIZES = (32, 16, 8, 4)   # slot-chunk granularities (binary tail)
CB = 64                        # gather-batch columns (CB*128 idxs per call)
MM = 32                        # columns per PSUM tile (4 banks)

_CACHE = {}


def _plan_core(entry_i, entry_w, counts):
    """Build per-chunk-size (idx, w, chunk_o) arrays for one core.

    Each output row's count c is decomposed as 32-chunks plus a tail of
    {16, 8, 4}-chunks with at most 3 padded slots:
      c = 32*n32 + r;  r = 16*t16 + 8*t8a + tail;  tail<=4 -> one 4-chunk,
      tail in 5..7 -> one 8-chunk.
    Returns dict L -> [idx_2d (n_chunks, L), w_2d, chunk_o (n_chunks,)].
    """
    n_o = counts.shape[0]
    starts = np.zeros(n_o + 1, dtype=np.int64)
    np.cumsum(counts, out=starts[1:])
    n32 = counts // 32
    r = counts - 32 * n32
    t16 = (r >= 16).astype(np.int64)
    r2 = r - 16 * t16
    t8a = (r2 >= 8).astype(np.int64)
    r3 = r2 - 8 * t8a
    t8b = (r3 >= 5).astype(np.int64)
    t4 = ((r3 >= 1) & (r3 <= 4)).astype(np.int64)

    n_per = {32: n32, 16: t16, 8: t8a + t8b, 4: t4}

    ranks = np.arange(entry_i.shape[0], dtype=np.int64)
    o_of_entry = np.repeat(np.arange(n_o, dtype=np.int64), counts)
    q = ranks - starts[o_of_entry]           # rank within o

    b0 = 32 * n32
    b1 = b0 + 16 * t16
    b2 = b1 + 8 * t8a

    out = {}
    for L in (32, 16, 8, 4):
        nL = n_per[L]
        n_chunks = int(nL.sum())
        chunk_start = np.zeros(n_o + 1, dtype=np.int64)
        np.cumsum(nL, out=chunk_start[1:])
        oe = o_of_entry
        if L == 32:
            sel = q < b0[oe]
            local = q[sel] - 0
            row = chunk_start[oe[sel]] + local // 32
            col = local % 32
        elif L == 16:
            sel = (q >= b0[oe]) & (q < b1[oe])
            row = chunk_start[oe[sel]]
            col = q[sel] - b0[oe[sel]]
        elif L == 8:
            sel_a = (q >= b1[oe]) & (q < b2[oe])
            sel_b = (q >= b2[oe]) & (t8b[oe] == 1)
            sel = sel_a | sel_b
            row = np.where(
                (q[sel] < b2[oe[sel]]),
                chunk_start[oe[sel]],
                chunk_start[oe[sel]] + t8a[oe[sel]],
            )
            col = np.where(
                q[sel] < b2[oe[sel]],
                q[sel] - b1[oe[sel]],
                q[sel] - b2[oe[sel]],
            )
        else:  # L == 4
            sel = (q >= b2[oe]) & (t4[oe] == 1)
            row = chunk_start[oe[sel]]
            col = q[sel] - b2[oe[sel]]
        idx2 = np.zeros((n_chunks, L), dtype=np.int64)
        w2 = np.zeros((n_chunks, L), dtype=np.float32)
        idx2[row, col] = entry_i[sel]
        w2[row, col] = entry_w[sel]
        chunk_o = np.repeat(np.arange(n_o, dtype=np.int64), nL)
        out[L] = [idx2, w2, chunk_o]
    return out


def _pack_stream(idx2, w2, L, n_cols):
    """Pack (n_chunks, L) chunk arrays into slot-stream matrices.

    Column c holds chunks [c*(P//L), (c+1)*(P//L)); chunk j -> partitions
    [j*L, (j+1)*L).  Returns:
      pair_idx (P, n_cols) int16  -- xT pair-row index (i >> 1)
      w_half   (P, n_cols, 2) f32 -- weight in half (i & 1), 0 in the other
    """
    g = P // L
    n_chunks = idx2.shape[0]
    idx_full = np.zeros((n_cols * g, L), dtype=np.int64)
    w_full = np.zeros((n_cols * g, L), dtype=np.float32)
    idx_full[:n_chunks] = idx2
    w_full[:n_chunks] = w2
    # column c, partition p = j*L+s  ->  chunk c*g + j, slot s
    idxm = idx_full.reshape(n_cols, g * L).T      # (P, n_cols)
    wm = w_full.reshape(n_cols, g * L).T
    pair_idx = (idxm >> 1).astype(np.int16)
    parity = (idxm & 1).astype(np.int64)
    w_half = np.zeros((P, n_cols, 2), dtype=np.float32)
    pp, cc = np.meshgrid(np.arange(P), np.arange(n_cols), indexing="ij")
    w_half[pp, cc, parity] = wm
    return np.ascontiguousarray(pair_idx), np.ascontiguousarray(w_half)


def _wrap_idxs(pair_idx):
    """(P, C) slot-stream -> dma_gather idx tile (128, C*8) int16.

    Flat gather index k = c*128 + p; the gather reads idx k from
    [partition k % 16, column k // 16], replicated x8 over 128 partitions.
    """
    flat = pair_idx.T.reshape(-1)                 # k = c*128 + p
    wrapped = flat.reshape(-1, 16).T              # (16, C*8)
    return np.ascontiguousarray(np.tile(wrapped, (8, 1)))


def _build_program(n_calls_by_L, repeat=1, bufs=(3, 2, 3)):
    """Construct the Bass SPMD program. n_calls_by_L: {L: n_cols} (uniform
    across cores; multiples of CB). repeat>1 re-traces the whole body
    (timing amplification; outputs are simply overwritten)."""
    import concourse.bass as bass
    import concourse.mybir as mybir
    from concourse import tile
    from concourse import library_config

    _apply_tile_patches()

    nc = bass.Bass(num_swdge_queues=2)
    table = nc.declare_dram_parameter(
        "xT2", [I // 2, 2 * B], mybir.dt.float32, isOutput=False
    )
    idx_p = {}
    w_p = {}
    out_p = {}
    for L, n_cols in n_calls_by_L.items():
        if n_cols == 0:
            continue
        nb = P // L
        idx_p[L] = nc.declare_dram_parameter(
            f"idx{L}", [P, n_cols * 8], mybir.dt.int16, isOutput=False
        )
        w_p[L] = nc.declare_dram_parameter(
            f"w{L}", [P, n_cols * 2], mybir.dt.float32, isOutput=False
        )
        out_p[L] = nc.declare_dram_parameter(
            f"out{L}", [nb, n_cols * 2 * B], mybir.dt.float32, isOutput=True
        )

    with tile.TileContext(nc) as tc:
        with (
            tc.tile_pool(name="meta", bufs=1) as meta_pool,
            tc.tile_pool(name="gath", bufs=bufs[0]) as gath_pool,
            tc.tile_pool(name="psum", bufs=bufs[1], space="PSUM") as psum_pool,
            tc.tile_pool(name="outs", bufs=bufs[2]) as out_pool,
        ):
            nc.gpsimd.load_library(library_config.mlp)
            nreg = nc.gpsimd.to_reg(CB * P)
            ones_t = {}
            idx_t = {}
            w_t = {}
            for L, n_cols in n_calls_by_L.items():
                if n_cols == 0:
                    continue
                nb = P // L
                ones = meta_pool.tile([P, nb], mybir.dt.float32, tag=f"ones{L}")
                # block-ones: ones[p, j] = 1 iff p // L == j
                nc.gpsimd.memset(ones[:], 1.0)
                nc.gpsimd.affine_select(
                    out=ones[:], in_=ones[:],
                    compare_op=mybir.AluOpType.is_ge, fill=0.0,
                    base=0, pattern=[[-L, nb]], channel_multiplier=1,
                )
                nc.gpsimd.affine_select(
                    out=ones[:], in_=ones[:],
                    compare_op=mybir.AluOpType.is_ge, fill=0.0,
                    base=L - 1, pattern=[[L, nb]], channel_multiplier=-1,
                )
                ones_t[L] = ones
                it = meta_pool.tile(
                    [P, n_cols * 8], mybir.dt.int16, tag=f"idx{L}"
                )
                wt = meta_pool.tile(
                    [P, n_cols, 2], mybir.dt.float32, tag=f"w{L}"
                )
                nc.sync.dma_start(out=it[:], in_=idx_p[L][:])
                nc.sync.dma_start(
                    out=wt[:], in_=w_p[L][:].rearrange("p (c h) -> p c h", h=2)
                )
                idx_t[L] = it
                w_t[L] = wt

            for _rep in range(repeat):
              for L, n_cols in n_calls_by_L.items():
                if n_cols == 0:
                    continue
                nb = P // L
                n_batches = n_cols // CB
                for bi in range(n_batches):
                    gt = gath_pool.tile(
                        [P, CB, 2, B], mybir.dt.float32, tag="g"
                    )
                    nc.gpsimd.dma_gather(
                        out_ap=gt[:].rearrange("p c h b -> p c (h b)"),
                        in_ap=table[:],
                        idxs_ap=idx_t[L][:, bi * CB * 8:(bi + 1) * CB * 8],
                        num_idxs=CB * P,
                        num_idxs_reg=nreg,
                        elem_size=2 * B,
                        queue_num=bi % 2,
                        single_packet=False,
                    )
                    # multiply by weights (broadcast along B; wrong pair
                    # half has weight 0)
                    nc.vector.tensor_tensor(
                        out=gt[:],
                        in0=gt[:],
                        in1=w_t[L][:, bi * CB:(bi + 1) * CB, :]
                        .unsqueeze(3).broadcast_to([P, CB, 2, B]),
                        op=mybir.AluOpType.mult,
                    )
                    stage = out_pool.tile(
                        [nb, CB, 2, B], mybir.dt.float32, tag="ob"
                    )
                    for half in range(CB // MM):
                        ps = psum_pool.tile(
                            [nb, MM, 2, B], mybir.dt.float32, tag="ps",
                            space="PSUM",
                        )
                        for q in range(MM // 8):
                            c0 = half * MM + q * 8
                            nc.tensor.matmul(
                                out=ps[:, q * 8:(q + 1) * 8, :, :]
                                .rearrange("n c h b -> n (c h b)"),
                                lhsT=ones_t[L][:],
                                rhs=gt[:, c0:c0 + 8, :, :]
                                .rearrange("p c h b -> p (c h b)"),
                                start=True,
                                stop=True,
                            )
                        # PSUM -> SBUF stage (pair halves folded on host)
                        nc.scalar.copy(
                            out=stage[:, half * MM:(half + 1) * MM, :, :],
                            in_=ps[:],
                        )
                    nc.sync.dma_start(
                        out=out_p[L][:, bi * CB * 2 * B:(bi + 1) * CB * 2 * B],
                        in_=stage[:].rearrange("n c h b -> n (c h b)"),
                    )

    _legalize_waits(nc)
    mybir.codegen_inst_isa_subclasses(nc)
    return nc


def _prepare(x, forward_weights, input_mapping):
    """Host-side planning: returns (in_maps, assembly_meta, n_calls_by_L)."""
    xT = np.ascontiguousarray(np.asarray(x).T).astype(np.float32)  # (I, B)
    xT2 = xT.reshape(I // 2, 2 * B)
    o_all = np.asarray(input_mapping).reshape(-1).astype(np.int64)
    w_all = np.asarray(forward_weights).reshape(-1).astype(np.float32)
    i_all = np.arange(o_all.shape[0], dtype=np.int64) >> 5

    order = np.argsort(o_all, kind="stable")
    o_s = o_all[order]
    i_s = i_all[order]
    w_s = w_all[order]
    counts_full = np.bincount(o_all, minlength=O)

    core_plans = []
    pos = 0
    for c in range(N_CORES):
        o0 = c * O_PER_CORE
        counts = counts_full[o0:o0 + O_PER_CORE]
        n_ent = int(counts.sum())
        plan = _plan_core(
            i_s[pos:pos + n_ent], w_s[pos:pos + n_ent], counts
        )
        core_plans.append(plan)
        pos += n_ent

    n_calls_by_L = {}
    for L in CHUNK_SIZES:
        g = P // L
        max_chunks = max(p[L][0].shape[0] for p in core_plans)
        n_cols = -(-max_chunks // g)
        n_cols = -(-n_cols // CB) * CB   # multiple of CB
        n_calls_by_L[L] = n_cols

    in_maps = []
    metas = []
    for c in range(N_CORES):
        m = {"xT2": xT2}
        meta = {}
        for L in CHUNK_SIZES:
            idx2, w2, chunk_o = core_plans[c][L]
            n_cols = n_calls_by_L[L]
            if n_cols == 0:
                continue
            pair_idx, w_half = _pack_stream(idx2, w2, L, n_cols)
            m[f"idx{L}"] = _wrap_idxs(pair_idx)
            m[f"w{L}"] = w_half.reshape(P, n_cols * 2)
            meta[L] = (chunk_o, idx2.shape[0])
        in_maps.append(m)
        metas.append(meta)
    return in_maps, metas, n_calls_by_L


def _assemble(results, metas, n_calls_by_L):
    """Decode device outputs and accumulate per-chunk sums into (B, O)."""
    out = np.zeros((O, B), dtype=np.float32)
    for c in range(N_CORES):
        o0 = c * O_PER_CORE
        for L in CHUNK_SIZES:
            n_cols = n_calls_by_L[L]
            if n_cols == 0:
                continue
            chunk_o, n_chunks = metas[c][L]
            if n_chunks == 0:
                continue
            nb = P // L
            raw = results[c][f"out{L}"]           # (nb, n_cols*2*B)
            # [j, ((cc*2)+h)*B + b]: half-h partial sum of chunk cc*nb + j
            raw = raw.reshape(nb, n_cols, 2, B).sum(axis=2)
            sums = np.transpose(raw, (1, 0, 2)).reshape(n_cols * nb, B)
            np.add.at(out, chunk_o[:n_chunks] + o0, sums[:n_chunks])
    return np.ascontiguousarray(out.T)


def kernel(x, forward_weights, input_mapping, output_size):
    from concourse.bass_utils import run_bass_kernel_spmd

    x = np.asarray(x)
    forward_weights = np.asarray(forward_weights)
    input_mapping = np.asarray(input_mapping)
    assert int(output_size) == O

    in_maps, metas, n_calls_by_L = _prepare(x, forward_weights, input_mapping)

    key = tuple(sorted(n_calls_by_L.items()))
    if key not in _CACHE:
        _CACHE[key] = _build_program(n_calls_by_L)
    nc = _CACHE[key]

    res = run_bass_kernel_spmd(
        nc, in_maps, core_ids=list(range(N_CORES))
    )
    return _assemble(res.results, metas, n_calls_by_L)


# revision 10
# speedup vs baseline: 3.5613x; 1.2332x over previous
"""Trainium2 Bass kernel for nn_DenSparseMatrix (segment_reduce).

out[b, o] = sum_{i,m: mapping[i,m]==o} weights[i,m] * x[b,i]
  x: (32, 65536) f32, weights/mapping: (65536, 32), output (32, 65536) f32.

Strategy (8 NeuronCores, full-input contract):
  * Host sorts the 2M (i,m) entries by output index o; cores are sharded by
    o-range (8192 outputs each) so no cross-core reduction is needed.
  * Each o's entry list is cut into fixed-size chunks (32/16/8/4 slots,
    zero padded).  Chunk slots are mapped onto (partition, column) cells of
    a flat slot stream; one bulk dma_gather instruction fetches thousands
    of x-rows at once (994ns fixed cost amortized vs. per-128-row indirect
    DMA), reading 256B *pair rows* of the xT table so the pair index fits
    the gather's int16 index type.  Host-prepared weights zero the unused
    half of each pair.
  * DVE multiplies gathered pairs by weights; the TensorEngine contracts
    each 128-partition group with a constant block-ones matrix into PSUM;
    the Activation engine folds the two pair-halves together into SBUF.
  * Host adds per-chunk partials into the final (B, O) output (cheap
    O(#chunks) assembly of device-computed sums).
"""

import numpy as np

# ---------------------------------------------------------------------------
# Tile framework compatibility patches (this walrus build allows only one
# sync-wait per instruction; TileContext can attach more).
# ---------------------------------------------------------------------------


def _apply_tile_patches():
    import concourse.tile as tile_mod
    from concourse import mybir
    from concourse.vector_clock import ScopedClock

    def _split_drain_and_barrier(self, tick_clock, wait_clock):
        nc = self.nc
        drain_inst = nc.sync.drain()
        wait_clock.add_sem_waits(
            drain_inst.ins, ScopedClock({None: tick_clock.global_clock})
        )
        si = drain_inst.ins.sync_info
        if si is not None and len(si.on_wait) > 1:
            waits = list(si.on_wait)
            si.on_wait.clear()
            si.on_wait.append(waits[0])
            for w in waits[1:]:
                extra = nc.sync.drain()
                esi = extra.ins.sync_info
                if esi is None:
                    extra.ins.sync_info = mybir.SyncInfo(
                        on_wait=[w], on_update=[]
                    )
                else:
                    esi.on_wait.append(w)
        nc.all_engine_barrier()
        assert self.sems is not None
        popped = nc._tile_sem_poison_stack.pop()
        assert popped is self._sem_poison
        nc.clear_and_free_semaphores(list(self.sems.allocated().values()))
        nc.all_engine_barrier()

    tile_mod.TileContext._drain_and_barrier = _split_drain_and_barrier


def _legalize_waits(nc):
    from concourse import mybir

    for bb_name, bass_bb in nc.bb_map.items():
        bb = bass_bb.bb
        insts = bb.instructions
        out = []
        changed = False
        for inst in insts:
            si = inst.sync_info
            if si is not None and si.on_wait is not None and len(si.on_wait) > 1:
                waits = list(si.on_wait)
                si.on_wait.clear()
                si.on_wait.append(waits[0])
                eng = nc.engines[inst.engine]
                for w in waits[1:]:
                    nop = eng.nop(nofuse=True, hint="wait_split")
                    cur_list = nc.cur_bb.bb.instructions
                    assert cur_list and cur_list[-1].name == nop.ins.name
                    cur_list.pop()
                    if nop.ins.sync_info is None:
                        nop.ins.sync_info = mybir.SyncInfo(
                            on_wait=[w], on_update=[]
                        )
                    else:
                        nop.ins.sync_info.on_wait.append(w)
                    out.append(nop.ins)
                changed = True
            out.append(inst)
        if changed:
            insts[:] = out


# ---------------------------------------------------------------------------
# Problem constants
# ---------------------------------------------------------------------------
B = 32          # batch
I = 65536       # input size
M = 32          # mapping width
O = 65536       # output size
N_CORES = 8
O_PER_CORE = O // N_CORES      # 8192
P = 128                        # SBUF partitions
CHUNK_SIZES = (32, 16, 8, 4)   # slot-chunk granularities (binary tail)
CB = 64                        # gather-batch columns (CB*128 idxs per call)
MM = 32                        # columns per PSUM tile (4 banks)

_CACHE = {}


def _plan_core(entry_i, entry_w, counts):
    """Build per-chunk-size (idx, w, chunk_o) arrays for one core.

    Each output row's count c is decomposed as 32-chunks plus a tail of
    {16, 8, 4}-chunks with at most 3 padded slots:
      c = 32*n32 + r;  r = 16*t16 + 8*t8a + tail;  tail<=4 -> one 4-chunk,
      tail in 5..7 -> one 8-chunk.
    Returns dict L -> [idx_2d (n_chunks, L), w_2d, chunk_o (n_chunks,)].
    """
    n_o = counts.shape[0]
    starts = np.zeros(n_o + 1, dtype=np.int64)
    np.cumsum(counts, out=starts[1:])
    n32 = counts // 32
    r = counts - 32 * n32
    t16 = (r >= 16).astype(np.int64)
    r2 = r - 16 * t16
    t8a = (r2 >= 8).astype(np.int64)
    r3 = r2 - 8 * t8a
    t8b = (r3 >= 5).astype(np.int64)
    t4 = ((r3 >= 1) & (r3 <= 4)).astype(np.int64)

    n_per = {32: n32, 16: t16, 8: t8a + t8b, 4: t4}

    ranks = np.arange(entry_i.shape[0], dtype=np.int64)
    o_of_entry = np.repeat(np.arange(n_o, dtype=np.int64), counts)
    q = ranks - starts[o_of_entry]           # rank within o

    b0 = 32 * n32
    b1 = b0 + 16 * t16
    b2 = b1 + 8 * t8a

    out = {}
    for L in (32, 16, 8, 4):
        nL = n_per[L]
        n_chunks = int(nL.sum())
        chunk_start = np.zeros(n_o + 1, dtype=np.int64)
        np.cumsum(nL, out=chunk_start[1:])
        oe = o_of_entry
        if L == 32:
            sel = q < b0[oe]
            local = q[sel] - 0
            row = chunk_start[oe[sel]] + local // 32
            col = local % 32
        elif L == 16:
            sel = (q >= b0[oe]) & (q < b1[oe])
            row = chunk_start[oe[sel]]
            col = q[sel] - b0[oe[sel]]
        elif L == 8:
            sel_a = (q >= b1[oe]) & (q < b2[oe])
            sel_b = (q >= b2[oe]) & (t8b[oe] == 1)
            sel = sel_a | sel_b
            row = np.where(
                (q[sel] < b2[oe[sel]]),
                chunk_start[oe[sel]],
                chunk_start[oe[sel]] + t8a[oe[sel]],
            )
            col = np.where(
                q[sel] < b2[oe[sel]],
                q[sel] - b1[oe[sel]],
                q[sel] - b2[oe[sel]],
            )
        else:  # L == 4
            sel = (q >= b2[oe]) & (t4[oe] == 1)
            row = chunk_start[oe[sel]]
            col = q[sel] - b2[oe[sel]]
        idx2 = np.zeros((n_chunks, L), dtype=np.int64)
        w2 = np.zeros((n_chunks, L), dtype=np.float32)
        idx2[row, col] = entry_i[sel]
        w2[row, col] = entry_w[sel]
        chunk_o = np.repeat(np.arange(n_o, dtype=np.int64), nL)
        out[L] = [idx2, w2, chunk_o]
    return out


def _pack_stream(idx2, w2, L, n_cols):
    """Pack (n_chunks, L) chunk arrays into slot-stream matrices.

    Column c holds chunks [c*(P//L), (c+1)*(P//L)); chunk j -> partitions
    [j*L, (j+1)*L).  Returns:
      pair_idx (P, n_cols) int16  -- xT pair-row index (i >> 1)
      w_half   (P, n_cols, 2) f32 -- weight in half (i & 1), 0 in the other
    """
    g = P // L
    n_chunks = idx2.shape[0]
    idx_full = np.zeros((n_cols * g, L), dtype=np.int64)
    w_full = np.zeros((n_cols * g, L), dtype=np.float32)
    idx_full[:n_chunks] = idx2
    w_full[:n_chunks] = w2
    # column c, partition p = j*L+s  ->  chunk c*g + j, slot s
    idxm = idx_full.reshape(n_cols, g * L).T      # (P, n_cols)
    wm = w_full.reshape(n_cols, g * L).T
    pair_idx = (idxm >> 1).astype(np.int16)
    parity = (idxm & 1).astype(np.int64)
    w_half = np.zeros((P, n_cols, 2), dtype=np.float32)
    pp, cc = np.meshgrid(np.arange(P), np.arange(n_cols), indexing="ij")
    w_half[pp, cc, parity] = wm
    return np.ascontiguousarray(pair_idx), np.ascontiguousarray(w_half)


def _wrap_idxs(pair_idx):
    """(P, C) slot-stream -> dma_gather idx tile (128, C*8) int16.

    Flat gather index k = c*128 + p; the gather reads idx k from
    [partition k % 16, column k // 16], replicated x8 over 128 partitions.
    """
    flat = pair_idx.T.reshape(-1)                 # k = c*128 + p
    wrapped = flat.reshape(-1, 16).T              # (16, C*8)
    return np.ascontiguousarray(np.tile(wrapped, (8, 1)))


def _build_program(n_calls_by_L, repeat=1, bufs=(3, 2, 3)):
    """Construct the Bass SPMD program. n_calls_by_L: {L: n_cols} (uniform
    across cores; multiples of CB). repeat>1 re-traces the whole body
    (timing amplification; outputs are simply overwritten)."""
    import concourse.bass as bass
    import concourse.mybir as mybir
    from concourse import tile
    from concourse import library_config

    _apply_tile_patches()

    nc = bass.Bass(num_swdge_queues=4)
    table = nc.declare_dram_parameter(
        "xT2", [I // 2, 2 * B], mybir.dt.float32, isOutput=False
    )
    idx_p = {}
    w_p = {}
    out_p = {}
    for L, n_cols in n_calls_by_L.items():
        if n_cols == 0:
            continue
        nb = P // L
        idx_p[L] = nc.declare_dram_parameter(
            f"idx{L}", [P, n_cols * 8], mybir.dt.int16, isOutput=False
        )
        w_p[L] = nc.declare_dram_parameter(
            f"w{L}", [P, n_cols * 2], mybir.dt.float32, isOutput=False
        )
        out_p[L] = nc.declare_dram_parameter(
            f"out{L}", [nb, n_cols * 2 * B], mybir.dt.float32, isOutput=True
        )

    with tile.TileContext(nc) as tc:
        with (
            tc.tile_pool(name="meta", bufs=1) as meta_pool,
            tc.tile_pool(name="gath", bufs=bufs[0]) as gath_pool,
            tc.tile_pool(name="psum", bufs=bufs[1], space="PSUM") as psum_pool,
            tc.tile_pool(name="outs", bufs=bufs[2]) as out_pool,
        ):
            nc.gpsimd.load_library(library_config.mlp)
            nreg = nc.gpsimd.to_reg(CB * P)
            ones_t = {}
            idx_t = {}
            w_t = {}
            for L, n_cols in n_calls_by_L.items():
                if n_cols == 0:
                    continue
                nb = P // L
                ones = meta_pool.tile([P, nb], mybir.dt.float32, tag=f"ones{L}")
                # block-ones: ones[p, j] = 1 iff p // L == j
                nc.gpsimd.memset(ones[:], 1.0)
                nc.gpsimd.affine_select(
                    out=ones[:], in_=ones[:],
                    compare_op=mybir.AluOpType.is_ge, fill=0.0,
                    base=0, pattern=[[-L, nb]], channel_multiplier=1,
                )
                nc.gpsimd.affine_select(
                    out=ones[:], in_=ones[:],
                    compare_op=mybir.AluOpType.is_ge, fill=0.0,
                    base=L - 1, pattern=[[L, nb]], channel_multiplier=-1,
                )
                ones_t[L] = ones
                it = meta_pool.tile(
                    [P, n_cols * 8], mybir.dt.int16, tag=f"idx{L}"
                )
                wt = meta_pool.tile(
                    [P, n_cols, 2], mybir.dt.float32, tag=f"w{L}"
                )
                nc.sync.dma_start(out=it[:], in_=idx_p[L][:])
                nc.sync.dma_start(
                    out=wt[:], in_=w_p[L][:].rearrange("p (c h) -> p c h", h=2)
                )
                idx_t[L] = it
                w_t[L] = wt

            for _rep in range(repeat):
              for L, n_cols in n_calls_by_L.items():
                if n_cols == 0:
                    continue
                nb = P // L
                n_batches = n_cols // CB
                for bi in range(n_batches):
                    gt = gath_pool.tile(
                        [P, CB, 2, B], mybir.dt.float32, tag="g"
                    )
                    nc.gpsimd.dma_gather(
                        out_ap=gt[:].rearrange("p c h b -> p c (h b)"),
                        in_ap=table[:],
                        idxs_ap=idx_t[L][:, bi * CB * 8:(bi + 1) * CB * 8],
                        num_idxs=CB * P,
                        num_idxs_reg=nreg,
                        elem_size=2 * B,
                        queue_num=bi % 4,
                        single_packet=False,
                    )
                    # multiply by weights (broadcast along B; wrong pair
                    # half has weight 0)
                    nc.vector.tensor_tensor(
                        out=gt[:],
                        in0=gt[:],
                        in1=w_t[L][:, bi * CB:(bi + 1) * CB, :]
                        .unsqueeze(3).broadcast_to([P, CB, 2, B]),
                        op=mybir.AluOpType.mult,
                    )
                    stage = out_pool.tile(
                        [nb, CB, 2, B], mybir.dt.float32, tag="ob"
                    )
                    for half in range(CB // MM):
                        ps = psum_pool.tile(
                            [nb, MM, 2, B], mybir.dt.float32, tag="ps",
                            space="PSUM",
                        )
                        for q in range(MM // 8):
                            c0 = half * MM + q * 8
                            nc.tensor.matmul(
                                out=ps[:, q * 8:(q + 1) * 8, :, :]
                                .rearrange("n c h b -> n (c h b)"),
                                lhsT=ones_t[L][:],
                                rhs=gt[:, c0:c0 + 8, :, :]
                                .rearrange("p c h b -> p (c h b)"),
                                start=True,
                                stop=True,
                            )
                        # PSUM -> SBUF stage (pair halves folded on host)
                        nc.scalar.copy(
                            out=stage[:, half * MM:(half + 1) * MM, :, :],
                            in_=ps[:],
                        )
                    nc.sync.dma_start(
                        out=out_p[L][:, bi * CB * 2 * B:(bi + 1) * CB * 2 * B],
                        in_=stage[:].rearrange("n c h b -> n (c h b)"),
                    )

    _legalize_waits(nc)
    mybir.codegen_inst_isa_subclasses(nc)
    return nc


def _prepare(x, forward_weights, input_mapping):
    """Host-side planning: returns (in_maps, assembly_meta, n_calls_by_L)."""
    xT = np.ascontiguousarray(np.asarray(x).T).astype(np.float32)  # (I, B)
    xT2 = xT.reshape(I // 2, 2 * B)
    o_all = np.asarray(input_mapping).reshape(-1).astype(np.int64)
    w_all = np.asarray(forward_weights).reshape(-1).astype(np.float32)
    i_all = np.arange(o_all.shape[0], dtype=np.int64) >> 5

    order = np.argsort(o_all, kind="stable")
    o_s = o_all[order]
    i_s = i_all[order]
    w_s = w_all[order]
    counts_full = np.bincount(o_all, minlength=O)

    core_plans = []
    pos = 0
    for c in range(N_CORES):
        o0 = c * O_PER_CORE
        counts = counts_full[o0:o0 + O_PER_CORE]
        n_ent = int(counts.sum())
        plan = _plan_core(
            i_s[pos:pos + n_ent], w_s[pos:pos + n_ent], counts
        )
        core_plans.append(plan)
        pos += n_ent

    n_calls_by_L = {}
    for L in CHUNK_SIZES:
        g = P // L
        max_chunks = max(p[L][0].shape[0] for p in core_plans)
        n_cols = -(-max_chunks // g)
        n_cols = -(-n_cols // CB) * CB   # multiple of CB
        n_calls_by_L[L] = n_cols

    in_maps = []
    metas = []
    for c in range(N_CORES):
        m = {"xT2": xT2}
        meta = {}
        for L in CHUNK_SIZES:
            idx2, w2, chunk_o = core_plans[c][L]
            n_cols = n_calls_by_L[L]
            if n_cols == 0:
                continue
            pair_idx, w_half = _pack_stream(idx2, w2, L, n_cols)
            m[f"idx{L}"] = _wrap_idxs(pair_idx)
            m[f"w{L}"] = w_half.reshape(P, n_cols * 2)
            meta[L] = (chunk_o, idx2.shape[0])
        in_maps.append(m)
        metas.append(meta)
    return in_maps, metas, n_calls_by_L


def _assemble(results, metas, n_calls_by_L):
    """Decode device outputs and accumulate per-chunk sums into (B, O)."""
    out = np.zeros((O, B), dtype=np.float32)
    for c in range(N_CORES):
        o0 = c * O_PER_CORE
        for L in CHUNK_SIZES:
            n_cols = n_calls_by_L[L]
            if n_cols == 0:
                continue
            chunk_o, n_chunks = metas[c][L]
            if n_chunks == 0:
                continue
            nb = P // L
            raw = results[c][f"out{L}"]           # (nb, n_cols*2*B)
            # [j, ((cc*2)+h)*B + b]: half-h partial sum of chunk cc*nb + j
            raw = raw.reshape(nb, n_cols, 2, B).sum(axis=2)
            sums = np.transpose(raw, (1, 0, 2)).reshape(n_cols * nb, B)
            np.add.at(out, chunk_o[:n_chunks] + o0, sums[:n_chunks])
    return np.ascontiguousarray(out.T)


def kernel(x, forward_weights, input_mapping, output_size):
    from concourse.bass_utils import run_bass_kernel_spmd

    x = np.asarray(x)
    forward_weights = np.asarray(forward_weights)
    input_mapping = np.asarray(input_mapping)
    assert int(output_size) == O

    in_maps, metas, n_calls_by_L = _prepare(x, forward_weights, input_mapping)

    key = tuple(sorted(n_calls_by_L.items()))
    if key not in _CACHE:
        _CACHE[key] = _build_program(n_calls_by_L)
    nc = _CACHE[key]

    res = run_bass_kernel_spmd(
        nc, in_maps, core_ids=list(range(N_CORES))
    )
    return _assemble(res.results, metas, n_calls_by_L)


# revision 11
# speedup vs baseline: 13.5820x; 3.8137x over previous
"""Trainium2 Bass kernel for nn_DenSparseMatrix (segment_reduce).

out[b, o] = sum_{i,m: mapping[i,m]==o} weights[i,m] * x[b,i]
  x: (32, 65536) f32, weights/mapping: (65536, 32), output (32, 65536) f32.

Strategy (8 NeuronCores, full-input contract):
  * Host sorts the 2M (i,m) entries by output index o; cores are sharded by
    o-range (8192 outputs each) so no cross-core reduction is needed.
  * Each o's entry list is cut into fixed-size chunks (32/16/8/4 slots,
    zero padded).  Chunk slots are mapped onto (partition, column) cells of
    a flat slot stream; one bulk dma_gather instruction fetches thousands
    of x-rows at once (994ns fixed cost amortized vs. per-128-row indirect
    DMA), reading 256B *pair rows* of the xT table so the pair index fits
    the gather's int16 index type.  Host-prepared weights zero the unused
    half of each pair.
  * DVE multiplies gathered pairs by weights; the TensorEngine contracts
    each 128-partition group with a constant block-ones matrix into PSUM;
    the Activation engine folds the two pair-halves together into SBUF.
  * Host adds per-chunk partials into the final (B, O) output (cheap
    O(#chunks) assembly of device-computed sums).
"""

import numpy as np

# ---------------------------------------------------------------------------
# Tile framework compatibility patches (this walrus build allows only one
# sync-wait per instruction; TileContext can attach more).
# ---------------------------------------------------------------------------


def _apply_tile_patches():
    import concourse.tile as tile_mod
    from concourse import mybir
    from concourse.vector_clock import ScopedClock

    def _split_drain_and_barrier(self, tick_clock, wait_clock):
        nc = self.nc
        drain_inst = nc.sync.drain()
        wait_clock.add_sem_waits(
            drain_inst.ins, ScopedClock({None: tick_clock.global_clock})
        )
        si = drain_inst.ins.sync_info
        if si is not None and len(si.on_wait) > 1:
            waits = list(si.on_wait)
            si.on_wait.clear()
            si.on_wait.append(waits[0])
            for w in waits[1:]:
                extra = nc.sync.drain()
                esi = extra.ins.sync_info
                if esi is None:
                    extra.ins.sync_info = mybir.SyncInfo(
                        on_wait=[w], on_update=[]
                    )
                else:
                    esi.on_wait.append(w)
        nc.all_engine_barrier()
        assert self.sems is not None
        popped = nc._tile_sem_poison_stack.pop()
        assert popped is self._sem_poison
        nc.clear_and_free_semaphores(list(self.sems.allocated().values()))
        nc.all_engine_barrier()

    tile_mod.TileContext._drain_and_barrier = _split_drain_and_barrier


def _legalize_waits(nc):
    from concourse import mybir

    for bb_name, bass_bb in nc.bb_map.items():
        bb = bass_bb.bb
        insts = bb.instructions
        out = []
        changed = False
        for inst in insts:
            si = inst.sync_info
            if si is not None and si.on_wait is not None and len(si.on_wait) > 1:
                waits = list(si.on_wait)
                si.on_wait.clear()
                si.on_wait.append(waits[0])
                eng = nc.engines[inst.engine]
                for w in waits[1:]:
                    nop = eng.nop(nofuse=True, hint="wait_split")
                    cur_list = nc.cur_bb.bb.instructions
                    assert cur_list and cur_list[-1].name == nop.ins.name
                    cur_list.pop()
                    if nop.ins.sync_info is None:
                        nop.ins.sync_info = mybir.SyncInfo(
                            on_wait=[w], on_update=[]
                        )
                    else:
                        nop.ins.sync_info.on_wait.append(w)
                    out.append(nop.ins)
                changed = True
            out.append(inst)
        if changed:
            insts[:] = out


# ---------------------------------------------------------------------------
# Problem constants
# ---------------------------------------------------------------------------
B = 32          # batch
I = 65536       # input size
M = 32          # mapping width
O = 65536       # output size
N_CORES = 8
O_PER_CORE = O // N_CORES      # 8192
P = 128                        # SBUF partitions
CHUNK_SIZES = (32, 16, 8, 4)   # slot-chunk granularities (binary tail)
CB = 32                        # gather-batch columns (CB*128 idxs per call)
MM = 32                        # columns per PSUM tile (4 banks)

_CACHE = {}


def _plan_core(entry_i, entry_w, counts):
    """Build per-chunk-size (idx, w, chunk_o) arrays for one core.

    Each output row's count c is decomposed as 32-chunks plus a tail of
    {16, 8, 4}-chunks with at most 3 padded slots:
      c = 32*n32 + r;  r = 16*t16 + 8*t8a + tail;  tail<=4 -> one 4-chunk,
      tail in 5..7 -> one 8-chunk.
    Returns dict L -> [idx_2d (n_chunks, L), w_2d, chunk_o (n_chunks,)].
    """
    n_o = counts.shape[0]
    starts = np.zeros(n_o + 1, dtype=np.int64)
    np.cumsum(counts, out=starts[1:])
    n32 = counts // 32
    r = counts - 32 * n32
    t16 = (r >= 16).astype(np.int64)
    r2 = r - 16 * t16
    t8a = (r2 >= 8).astype(np.int64)
    r3 = r2 - 8 * t8a
    t8b = (r3 >= 5).astype(np.int64)
    t4 = ((r3 >= 1) & (r3 <= 4)).astype(np.int64)

    n_per = {32: n32, 16: t16, 8: t8a + t8b, 4: t4}

    ranks = np.arange(entry_i.shape[0], dtype=np.int64)
    o_of_entry = np.repeat(np.arange(n_o, dtype=np.int64), counts)
    q = ranks - starts[o_of_entry]           # rank within o

    b0 = 32 * n32
    b1 = b0 + 16 * t16
    b2 = b1 + 8 * t8a

    out = {}
    for L in (32, 16, 8, 4):
        nL = n_per[L]
        n_chunks = int(nL.sum())
        chunk_start = np.zeros(n_o + 1, dtype=np.int64)
        np.cumsum(nL, out=chunk_start[1:])
        oe = o_of_entry
        if L == 32:
            sel = q < b0[oe]
            local = q[sel] - 0
            row = chunk_start[oe[sel]] + local // 32
            col = local % 32
        elif L == 16:
            sel = (q >= b0[oe]) & (q < b1[oe])
            row = chunk_start[oe[sel]]
            col = q[sel] - b0[oe[sel]]
        elif L == 8:
            sel_a = (q >= b1[oe]) & (q < b2[oe])
            sel_b = (q >= b2[oe]) & (t8b[oe] == 1)
            sel = sel_a | sel_b
            row = np.where(
                (q[sel] < b2[oe[sel]]),
                chunk_start[oe[sel]],
                chunk_start[oe[sel]] + t8a[oe[sel]],
            )
            col = np.where(
                q[sel] < b2[oe[sel]],
                q[sel] - b1[oe[sel]],
                q[sel] - b2[oe[sel]],
            )
        else:  # L == 4
            sel = (q >= b2[oe]) & (t4[oe] == 1)
            row = chunk_start[oe[sel]]
            col = q[sel] - b2[oe[sel]]
        idx2 = np.zeros((n_chunks, L), dtype=np.int64)
        w2 = np.zeros((n_chunks, L), dtype=np.float32)
        idx2[row, col] = entry_i[sel]
        w2[row, col] = entry_w[sel]
        chunk_o = np.repeat(np.arange(n_o, dtype=np.int64), nL)
        out[L] = [idx2, w2, chunk_o]
    return out


def _pack_stream(idx2, w2, L, n_cols):
    """Pack (n_chunks, L) chunk arrays into slot-stream matrices.

    Column c holds chunks [c*(P//L), (c+1)*(P//L)); chunk j -> partitions
    [j*L, (j+1)*L).  Returns:
      pair_idx (P, n_cols) int16  -- xT pair-row index (i >> 1)
      w_half   (P, n_cols, 2) f32 -- weight in half (i & 1), 0 in the other
    """
    g = P // L
    n_chunks = idx2.shape[0]
    idx_full = np.zeros((n_cols * g, L), dtype=np.int64)
    w_full = np.zeros((n_cols * g, L), dtype=np.float32)
    idx_full[:n_chunks] = idx2
    w_full[:n_chunks] = w2
    # column c, partition p = j*L+s  ->  chunk c*g + j, slot s
    idxm = idx_full.reshape(n_cols, g * L).T      # (P, n_cols)
    wm = w_full.reshape(n_cols, g * L).T
    pair_idx = (idxm >> 1).astype(np.int16)
    parity = (idxm & 1).astype(np.int64)
    w_half = np.zeros((P, n_cols, 2), dtype=np.float32)
    pp, cc = np.meshgrid(np.arange(P), np.arange(n_cols), indexing="ij")
    w_half[pp, cc, parity] = wm
    return np.ascontiguousarray(pair_idx), np.ascontiguousarray(w_half)


def _wrap_idxs(pair_idx):
    """(P, C) slot-stream -> dma_gather idx tile (128, C*8) int16.

    Flat gather index k = c*128 + p; the gather reads idx k from
    [partition k % 16, column k // 16], replicated x8 over 128 partitions.
    """
    flat = pair_idx.T.reshape(-1)                 # k = c*128 + p
    wrapped = flat.reshape(-1, 16).T              # (16, C*8)
    return np.ascontiguousarray(np.tile(wrapped, (8, 1)))


def _build_program(n_calls_by_L, repeat=1, bufs=(6, 2, 3)):
    """Construct the Bass SPMD program. n_calls_by_L: {L: n_cols} (uniform
    across cores; multiples of CB). repeat>1 re-traces the whole body
    (timing amplification; outputs are simply overwritten)."""
    import concourse.bass as bass
    import concourse.mybir as mybir
    from concourse import tile
    from concourse import library_config

    _apply_tile_patches()

    nc = bass.Bass(num_swdge_queues=4)
    table = nc.declare_dram_parameter(
        "xT2", [I // 2, 2 * B], mybir.dt.float32, isOutput=False
    )
    idx_p = {}
    w_p = {}
    out_p = {}
    for L, n_cols in n_calls_by_L.items():
        if n_cols == 0:
            continue
        nb = P // L
        idx_p[L] = nc.declare_dram_parameter(
            f"idx{L}", [P, n_cols * 8], mybir.dt.int16, isOutput=False
        )
        w_p[L] = nc.declare_dram_parameter(
            f"w{L}", [P, n_cols * 2], mybir.dt.float32, isOutput=False
        )
        out_p[L] = nc.declare_dram_parameter(
            f"out{L}", [nb, n_cols * 2 * B], mybir.dt.float32, isOutput=True
        )

    with tile.TileContext(nc) as tc:
        with (
            tc.tile_pool(name="meta", bufs=1) as meta_pool,
            tc.tile_pool(name="gath", bufs=bufs[0]) as gath_pool,
            tc.tile_pool(name="psum", bufs=bufs[1], space="PSUM") as psum_pool,
            tc.tile_pool(name="outs", bufs=bufs[2]) as out_pool,
        ):
            nc.gpsimd.load_library(library_config.mlp)
            nreg = nc.gpsimd.to_reg(CB * P)
            ones_t = {}
            idx_t = {}
            w_t = {}
            for L, n_cols in n_calls_by_L.items():
                if n_cols == 0:
                    continue
                nb = P // L
                ones = meta_pool.tile([P, nb], mybir.dt.float32, tag=f"ones{L}")
                # block-ones: ones[p, j] = 1 iff p // L == j
                nc.gpsimd.memset(ones[:], 1.0)
                nc.gpsimd.affine_select(
                    out=ones[:], in_=ones[:],
                    compare_op=mybir.AluOpType.is_ge, fill=0.0,
                    base=0, pattern=[[-L, nb]], channel_multiplier=1,
                )
                nc.gpsimd.affine_select(
                    out=ones[:], in_=ones[:],
                    compare_op=mybir.AluOpType.is_ge, fill=0.0,
                    base=L - 1, pattern=[[L, nb]], channel_multiplier=-1,
                )
                ones_t[L] = ones
                it = meta_pool.tile(
                    [P, n_cols * 8], mybir.dt.int16, tag=f"idx{L}"
                )
                wt = meta_pool.tile(
                    [P, n_cols, 2], mybir.dt.float32, tag=f"w{L}"
                )
                nc.sync.dma_start(out=it[:], in_=idx_p[L][:])
                nc.sync.dma_start(
                    out=wt[:], in_=w_p[L][:].rearrange("p (c h) -> p c h", h=2)
                )
                idx_t[L] = it
                w_t[L] = wt

            qi = 0
            for _rep in range(repeat):
              for L, n_cols in n_calls_by_L.items():
                if n_cols == 0:
                    continue
                nb = P // L
                n_batches = n_cols // CB
                for bi in range(n_batches):
                    qi += 1
                    gt = gath_pool.tile(
                        [P, CB, 2, B], mybir.dt.float32, tag="g"
                    )
                    nc.gpsimd.dma_gather(
                        out_ap=gt[:].rearrange("p c h b -> p c (h b)"),
                        in_ap=table[:],
                        idxs_ap=idx_t[L][:, bi * CB * 8:(bi + 1) * CB * 8],
                        num_idxs=CB * P,
                        num_idxs_reg=nreg,
                        elem_size=2 * B,
                        queue_num=qi % 4,
                        single_packet=False,
                    )
                    # multiply by weights (broadcast along B; wrong pair
                    # half has weight 0)
                    nc.vector.tensor_tensor(
                        out=gt[:],
                        in0=gt[:],
                        in1=w_t[L][:, bi * CB:(bi + 1) * CB, :]
                        .unsqueeze(3).broadcast_to([P, CB, 2, B]),
                        op=mybir.AluOpType.mult,
                    )
                    stage = out_pool.tile(
                        [nb, CB, 2, B], mybir.dt.float32, tag="ob"
                    )
                    for half in range(CB // MM):
                        ps = psum_pool.tile(
                            [nb, MM, 2, B], mybir.dt.float32, tag="ps",
                            space="PSUM",
                        )
                        for q in range(MM // 8):
                            c0 = half * MM + q * 8
                            nc.tensor.matmul(
                                out=ps[:, q * 8:(q + 1) * 8, :, :]
                                .rearrange("n c h b -> n (c h b)"),
                                lhsT=ones_t[L][:],
                                rhs=gt[:, c0:c0 + 8, :, :]
                                .rearrange("p c h b -> p (c h b)"),
                                start=True,
                                stop=True,
                            )
                        # PSUM -> SBUF stage (pair halves folded on host)
                        nc.scalar.copy(
                            out=stage[:, half * MM:(half + 1) * MM, :, :],
                            in_=ps[:],
                        )
                    nc.sync.dma_start(
                        out=out_p[L][:, bi * CB * 2 * B:(bi + 1) * CB * 2 * B],
                        in_=stage[:].rearrange("n c h b -> n (c h b)"),
                    )

    _legalize_waits(nc)
    mybir.codegen_inst_isa_subclasses(nc)
    return nc


def _prepare(x, forward_weights, input_mapping):
    """Host-side planning: returns (in_maps, assembly_meta, n_calls_by_L)."""
    xT = np.ascontiguousarray(np.asarray(x).T).astype(np.float32)  # (I, B)
    xT2 = xT.reshape(I // 2, 2 * B)
    o_all = np.asarray(input_mapping).reshape(-1).astype(np.int64)
    w_all = np.asarray(forward_weights).reshape(-1).astype(np.float32)
    i_all = np.arange(o_all.shape[0], dtype=np.int64) >> 5

    order = np.argsort(o_all, kind="stable")
    o_s = o_all[order]
    i_s = i_all[order]
    w_s = w_all[order]
    counts_full = np.bincount(o_all, minlength=O)

    core_plans = []
    pos = 0
    for c in range(N_CORES):
        o0 = c * O_PER_CORE
        counts = counts_full[o0:o0 + O_PER_CORE]
        n_ent = int(counts.sum())
        plan = _plan_core(
            i_s[pos:pos + n_ent], w_s[pos:pos + n_ent], counts
        )
        core_plans.append(plan)
        pos += n_ent

    n_calls_by_L = {}
    for L in CHUNK_SIZES:
        g = P // L
        max_chunks = max(p[L][0].shape[0] for p in core_plans)
        n_cols = -(-max_chunks // g)
        n_cols = -(-n_cols // CB) * CB   # multiple of CB
        n_calls_by_L[L] = n_cols

    in_maps = []
    metas = []
    for c in range(N_CORES):
        m = {"xT2": xT2}
        meta = {}
        for L in CHUNK_SIZES:
            idx2, w2, chunk_o = core_plans[c][L]
            n_cols = n_calls_by_L[L]
            if n_cols == 0:
                continue
            pair_idx, w_half = _pack_stream(idx2, w2, L, n_cols)
            m[f"idx{L}"] = _wrap_idxs(pair_idx)
            m[f"w{L}"] = w_half.reshape(P, n_cols * 2)
            meta[L] = (chunk_o, idx2.shape[0])
        in_maps.append(m)
        metas.append(meta)
    return in_maps, metas, n_calls_by_L


def _assemble(results, metas, n_calls_by_L):
    """Decode device outputs and accumulate per-chunk sums into (B, O)."""
    out = np.zeros((O, B), dtype=np.float32)
    for c in range(N_CORES):
        o0 = c * O_PER_CORE
        for L in CHUNK_SIZES:
            n_cols = n_calls_by_L[L]
            if n_cols == 0:
                continue
            chunk_o, n_chunks = metas[c][L]
            if n_chunks == 0:
                continue
            nb = P // L
            raw = results[c][f"out{L}"]           # (nb, n_cols*2*B)
            # [j, ((cc*2)+h)*B + b]: half-h partial sum of chunk cc*nb + j
            raw = raw.reshape(nb, n_cols, 2, B).sum(axis=2)
            sums = np.transpose(raw, (1, 0, 2)).reshape(n_cols * nb, B)
            np.add.at(out, chunk_o[:n_chunks] + o0, sums[:n_chunks])
    return np.ascontiguousarray(out.T)


def kernel(x, forward_weights, input_mapping, output_size):
    from concourse.bass_utils import run_bass_kernel_spmd

    x = np.asarray(x)
    forward_weights = np.asarray(forward_weights)
    input_mapping = np.asarray(input_mapping)
    assert int(output_size) == O

    in_maps, metas, n_calls_by_L = _prepare(x, forward_weights, input_mapping)

    key = tuple(sorted(n_calls_by_L.items()))
    if key not in _CACHE:
        _CACHE[key] = _build_program(n_calls_by_L)
    nc = _CACHE[key]

    res = run_bass_kernel_spmd(
        nc, in_maps, core_ids=list(range(N_CORES))
    )
    return _assemble(res.results, metas, n_calls_by_L)


# revision 12
# speedup vs baseline: 28.7125x; 2.1140x over previous
"""Trainium2 Bass kernel for nn_DenSparseMatrix (segment_reduce).

out[b, o] = sum_{i,m: mapping[i,m]==o} weights[i,m] * x[b,i]
  x: (32, 65536) f32, weights/mapping: (65536, 32), output (32, 65536) f32.

Strategy (8 NeuronCores, full-input contract):
  * Host sorts the 2M (i,m) entries by output index o; cores are sharded by
    o-range (8192 outputs each) so no cross-core reduction is needed.
  * Each o's entry list is cut into fixed-size chunks (32/16/8/4 slots,
    zero padded).  Chunk slots are mapped onto (partition, column) cells of
    a flat slot stream; one bulk dma_gather instruction fetches thousands
    of x-rows at once (994ns fixed cost amortized vs. per-128-row indirect
    DMA), reading 256B *pair rows* of the xT table so the pair index fits
    the gather's int16 index type.  Host-prepared weights zero the unused
    half of each pair.
  * DVE multiplies gathered pairs by weights; the TensorEngine contracts
    each 128-partition group with a constant block-ones matrix into PSUM;
    the Activation engine folds the two pair-halves together into SBUF.
  * Host adds per-chunk partials into the final (B, O) output (cheap
    O(#chunks) assembly of device-computed sums).
"""

import numpy as np

# ---------------------------------------------------------------------------
# Tile framework compatibility patches (this walrus build allows only one
# sync-wait per instruction; TileContext can attach more).
# ---------------------------------------------------------------------------


def _apply_tile_patches():
    import concourse.tile as tile_mod
    from concourse import mybir
    from concourse.vector_clock import ScopedClock

    def _split_drain_and_barrier(self, tick_clock, wait_clock):
        nc = self.nc
        drain_inst = nc.sync.drain()
        wait_clock.add_sem_waits(
            drain_inst.ins, ScopedClock({None: tick_clock.global_clock})
        )
        si = drain_inst.ins.sync_info
        if si is not None and len(si.on_wait) > 1:
            waits = list(si.on_wait)
            si.on_wait.clear()
            si.on_wait.append(waits[0])
            for w in waits[1:]:
                extra = nc.sync.drain()
                esi = extra.ins.sync_info
                if esi is None:
                    extra.ins.sync_info = mybir.SyncInfo(
                        on_wait=[w], on_update=[]
                    )
                else:
                    esi.on_wait.append(w)
        nc.all_engine_barrier()
        assert self.sems is not None
        popped = nc._tile_sem_poison_stack.pop()
        assert popped is self._sem_poison
        nc.clear_and_free_semaphores(list(self.sems.allocated().values()))
        nc.all_engine_barrier()

    tile_mod.TileContext._drain_and_barrier = _split_drain_and_barrier


def _legalize_waits(nc):
    from concourse import mybir

    for bb_name, bass_bb in nc.bb_map.items():
        bb = bass_bb.bb
        insts = bb.instructions
        out = []
        changed = False
        for inst in insts:
            si = inst.sync_info
            if si is not None and si.on_wait is not None and len(si.on_wait) > 1:
                waits = list(si.on_wait)
                si.on_wait.clear()
                si.on_wait.append(waits[0])
                eng = nc.engines[inst.engine]
                for w in waits[1:]:
                    nop = eng.nop(nofuse=True, hint="wait_split")
                    cur_list = nc.cur_bb.bb.instructions
                    assert cur_list and cur_list[-1].name == nop.ins.name
                    cur_list.pop()
                    if nop.ins.sync_info is None:
                        nop.ins.sync_info = mybir.SyncInfo(
                            on_wait=[w], on_update=[]
                        )
                    else:
                        nop.ins.sync_info.on_wait.append(w)
                    out.append(nop.ins)
                changed = True
            out.append(inst)
        if changed:
            insts[:] = out


# ---------------------------------------------------------------------------
# Problem constants
# ---------------------------------------------------------------------------
B = 32          # batch
I = 65536       # input size
M = 32          # mapping width
O = 65536       # output size
N_CORES = 8
O_PER_CORE = O // N_CORES      # 8192
P = 128                        # SBUF partitions
CHUNK_SIZES = (32, 16, 8, 4)   # slot-chunk granularities (binary tail)
CB = 32                        # gather-batch columns (CB*128 idxs per call)
MM = 32                        # columns per PSUM tile (4 banks)

_CACHE = {}


def _plan_core(entry_i, entry_w, counts):
    """Build per-chunk-size (idx, w, chunk_o) arrays for one core.

    Each output row's count c is decomposed as 32-chunks plus a tail of
    {16, 8, 4}-chunks with at most 3 padded slots:
      c = 32*n32 + r;  r = 16*t16 + 8*t8a + tail;  tail<=4 -> one 4-chunk,
      tail in 5..7 -> one 8-chunk.
    Returns dict L -> [idx_2d (n_chunks, L), w_2d, chunk_o (n_chunks,)].
    """
    n_o = counts.shape[0]
    starts = np.zeros(n_o + 1, dtype=np.int64)
    np.cumsum(counts, out=starts[1:])
    n32 = counts // 32
    r = counts - 32 * n32
    t16 = (r >= 16).astype(np.int64)
    r2 = r - 16 * t16
    t8a = (r2 >= 8).astype(np.int64)
    r3 = r2 - 8 * t8a
    t8b = (r3 >= 5).astype(np.int64)
    t4 = ((r3 >= 1) & (r3 <= 4)).astype(np.int64)

    n_per = {32: n32, 16: t16, 8: t8a + t8b, 4: t4}

    ranks = np.arange(entry_i.shape[0], dtype=np.int64)
    o_of_entry = np.repeat(np.arange(n_o, dtype=np.int64), counts)
    q = ranks - starts[o_of_entry]           # rank within o

    b0 = 32 * n32
    b1 = b0 + 16 * t16
    b2 = b1 + 8 * t8a

    out = {}
    for L in (32, 16, 8, 4):
        nL = n_per[L]
        n_chunks = int(nL.sum())
        chunk_start = np.zeros(n_o + 1, dtype=np.int64)
        np.cumsum(nL, out=chunk_start[1:])
        oe = o_of_entry
        if L == 32:
            sel = q < b0[oe]
            local = q[sel] - 0
            row = chunk_start[oe[sel]] + local // 32
            col = local % 32
        elif L == 16:
            sel = (q >= b0[oe]) & (q < b1[oe])
            row = chunk_start[oe[sel]]
            col = q[sel] - b0[oe[sel]]
        elif L == 8:
            sel_a = (q >= b1[oe]) & (q < b2[oe])
            sel_b = (q >= b2[oe]) & (t8b[oe] == 1)
            sel = sel_a | sel_b
            row = np.where(
                (q[sel] < b2[oe[sel]]),
                chunk_start[oe[sel]],
                chunk_start[oe[sel]] + t8a[oe[sel]],
            )
            col = np.where(
                q[sel] < b2[oe[sel]],
                q[sel] - b1[oe[sel]],
                q[sel] - b2[oe[sel]],
            )
        else:  # L == 4
            sel = (q >= b2[oe]) & (t4[oe] == 1)
            row = chunk_start[oe[sel]]
            col = q[sel] - b2[oe[sel]]
        idx2 = np.zeros((n_chunks, L), dtype=np.int64)
        w2 = np.zeros((n_chunks, L), dtype=np.float32)
        idx2[row, col] = entry_i[sel]
        w2[row, col] = entry_w[sel]
        chunk_o = np.repeat(np.arange(n_o, dtype=np.int64), nL)
        out[L] = [idx2, w2, chunk_o]
    return out


def _pack_stream(idx2, w2, L, n_cols):
    """Pack (n_chunks, L) chunk arrays into slot-stream matrices.

    Column c holds chunks [c*(P//L), (c+1)*(P//L)); chunk j -> partitions
    [j*L, (j+1)*L).  Returns:
      pair_idx (P, n_cols) int16  -- xT pair-row index (i >> 1)
      w_half   (P, n_cols, 2) f32 -- weight in half (i & 1), 0 in the other
    """
    g = P // L
    n_chunks = idx2.shape[0]
    idx_full = np.zeros((n_cols * g, L), dtype=np.int64)
    w_full = np.zeros((n_cols * g, L), dtype=np.float32)
    idx_full[:n_chunks] = idx2
    w_full[:n_chunks] = w2
    # column c, partition p = j*L+s  ->  chunk c*g + j, slot s
    idxm = idx_full.reshape(n_cols, g * L).T      # (P, n_cols)
    wm = w_full.reshape(n_cols, g * L).T
    pair_idx = (idxm >> 1).astype(np.int16)
    parity = (idxm & 1).astype(np.int64)
    w_half = np.zeros((P, n_cols, 2), dtype=np.float32)
    pp, cc = np.meshgrid(np.arange(P), np.arange(n_cols), indexing="ij")
    w_half[pp, cc, parity] = wm
    return np.ascontiguousarray(pair_idx), np.ascontiguousarray(w_half)


def _wrap_idxs(pair_idx):
    """(P, C) slot-stream -> dma_gather idx tile (128, C*8) int16.

    Flat gather index k = c*128 + p; the gather reads idx k from
    [partition k % 16, column k // 16], replicated x8 over 128 partitions.
    """
    flat = pair_idx.T.reshape(-1)                 # k = c*128 + p
    wrapped = flat.reshape(-1, 16).T              # (16, C*8)
    return np.ascontiguousarray(np.tile(wrapped, (8, 1)))


def _build_program(n_calls_by_L, repeat=1, bufs=(8, 2, 3)):
    """Construct the Bass SPMD program. n_calls_by_L: {L: n_cols} (uniform
    across cores; multiples of CB). repeat>1 re-traces the whole body
    (timing amplification; outputs are simply overwritten)."""
    import concourse.bass as bass
    import concourse.mybir as mybir
    from concourse import tile
    from concourse import library_config

    _apply_tile_patches()

    nc = bass.Bass(num_swdge_queues=4)
    table = nc.declare_dram_parameter(
        "xT2", [I // 2, 2 * B], mybir.dt.float32, isOutput=False
    )
    idx_p = {}
    w_p = {}
    out_p = {}
    for L, n_cols in n_calls_by_L.items():
        if n_cols == 0:
            continue
        nb = P // L
        idx_p[L] = nc.declare_dram_parameter(
            f"idx{L}", [P, n_cols * 8], mybir.dt.int16, isOutput=False
        )
        w_p[L] = nc.declare_dram_parameter(
            f"w{L}", [P, n_cols * 2], mybir.dt.float32, isOutput=False
        )
        out_p[L] = nc.declare_dram_parameter(
            f"out{L}", [nb, n_cols * 2 * B], mybir.dt.float32, isOutput=True
        )

    with tile.TileContext(nc) as tc:
        with (
            tc.tile_pool(name="meta", bufs=1) as meta_pool,
            tc.tile_pool(name="gath", bufs=bufs[0]) as gath_pool,
            tc.tile_pool(name="psum", bufs=bufs[1], space="PSUM") as psum_pool,
            tc.tile_pool(name="outs", bufs=bufs[2]) as out_pool,
        ):
            nc.gpsimd.load_library(library_config.mlp)
            nreg = nc.gpsimd.to_reg(CB * P)
            ones_t = {}
            idx_t = {}
            w_t = {}
            for L, n_cols in n_calls_by_L.items():
                if n_cols == 0:
                    continue
                nb = P // L
                ones = meta_pool.tile([P, nb], mybir.dt.float32, tag=f"ones{L}")
                # block-ones: ones[p, j] = 1 iff p // L == j
                nc.gpsimd.memset(ones[:], 1.0)
                nc.gpsimd.affine_select(
                    out=ones[:], in_=ones[:],
                    compare_op=mybir.AluOpType.is_ge, fill=0.0,
                    base=0, pattern=[[-L, nb]], channel_multiplier=1,
                )
                nc.gpsimd.affine_select(
                    out=ones[:], in_=ones[:],
                    compare_op=mybir.AluOpType.is_ge, fill=0.0,
                    base=L - 1, pattern=[[L, nb]], channel_multiplier=-1,
                )
                ones_t[L] = ones
                it = meta_pool.tile(
                    [P, n_cols * 8], mybir.dt.int16, tag=f"idx{L}"
                )
                wt = meta_pool.tile(
                    [P, n_cols, 2], mybir.dt.float32, tag=f"w{L}"
                )
                nc.sync.dma_start(out=it[:], in_=idx_p[L][:])
                nc.sync.dma_start(
                    out=wt[:], in_=w_p[L][:].rearrange("p (c h) -> p c h", h=2)
                )
                idx_t[L] = it
                w_t[L] = wt

            qi = 0
            for _rep in range(repeat):
              for L, n_cols in n_calls_by_L.items():
                if n_cols == 0:
                    continue
                nb = P // L
                n_batches = n_cols // CB
                for bi in range(n_batches):
                    qi += 1
                    gt = gath_pool.tile(
                        [P, CB, 2, B], mybir.dt.float32, tag="g"
                    )
                    nc.gpsimd.dma_gather(
                        out_ap=gt[:].rearrange("p c h b -> p c (h b)"),
                        in_ap=table[:],
                        idxs_ap=idx_t[L][:, bi * CB * 8:(bi + 1) * CB * 8],
                        num_idxs=CB * P,
                        num_idxs_reg=nreg,
                        elem_size=2 * B,
                        queue_num=qi % 4,
                        single_packet=False,
                    )
                    # multiply by weights (broadcast along B; wrong pair
                    # half has weight 0)
                    nc.vector.tensor_tensor(
                        out=gt[:],
                        in0=gt[:],
                        in1=w_t[L][:, bi * CB:(bi + 1) * CB, :]
                        .unsqueeze(3).broadcast_to([P, CB, 2, B]),
                        op=mybir.AluOpType.mult,
                    )
                    stage = out_pool.tile(
                        [nb, CB, 2, B], mybir.dt.float32, tag="ob"
                    )
                    for half in range(CB // MM):
                        ps = psum_pool.tile(
                            [nb, MM, 2, B], mybir.dt.float32, tag="ps",
                            space="PSUM",
                        )
                        for q in range(MM // 8):
                            c0 = half * MM + q * 8
                            nc.tensor.matmul(
                                out=ps[:, q * 8:(q + 1) * 8, :, :]
                                .rearrange("n c h b -> n (c h b)"),
                                lhsT=ones_t[L][:],
                                rhs=gt[:, c0:c0 + 8, :, :]
                                .rearrange("p c h b -> p (c h b)"),
                                start=True,
                                stop=True,
                            )
                        # PSUM -> SBUF stage (pair halves folded on host)
                        nc.scalar.copy(
                            out=stage[:, half * MM:(half + 1) * MM, :, :],
                            in_=ps[:],
                        )
                    nc.sync.dma_start(
                        out=out_p[L][:, bi * CB * 2 * B:(bi + 1) * CB * 2 * B],
                        in_=stage[:].rearrange("n c h b -> n (c h b)"),
                    )

    _legalize_waits(nc)
    mybir.codegen_inst_isa_subclasses(nc)
    return nc


def _prepare(x, forward_weights, input_mapping):
    """Host-side planning: returns (in_maps, assembly_meta, n_calls_by_L)."""
    xT = np.ascontiguousarray(np.asarray(x).T).astype(np.float32)  # (I, B)
    xT2 = xT.reshape(I // 2, 2 * B)
    o_all = np.asarray(input_mapping).reshape(-1).astype(np.int64)
    w_all = np.asarray(forward_weights).reshape(-1).astype(np.float32)
    i_all = np.arange(o_all.shape[0], dtype=np.int64) >> 5

    order = np.argsort(o_all, kind="stable")
    o_s = o_all[order]
    i_s = i_all[order]
    w_s = w_all[order]
    counts_full = np.bincount(o_all, minlength=O)

    core_plans = []
    pos = 0
    for c in range(N_CORES):
        o0 = c * O_PER_CORE
        counts = counts_full[o0:o0 + O_PER_CORE]
        n_ent = int(counts.sum())
        plan = _plan_core(
            i_s[pos:pos + n_ent], w_s[pos:pos + n_ent], counts
        )
        core_plans.append(plan)
        pos += n_ent

    n_calls_by_L = {}
    for L in CHUNK_SIZES:
        g = P // L
        max_chunks = max(p[L][0].shape[0] for p in core_plans)
        n_cols = -(-max_chunks // g)
        n_cols = -(-n_cols // CB) * CB   # multiple of CB
        n_calls_by_L[L] = n_cols

    in_maps = []
    metas = []
    for c in range(N_CORES):
        m = {"xT2": xT2}
        meta = {}
        for L in CHUNK_SIZES:
            idx2, w2, chunk_o = core_plans[c][L]
            n_cols = n_calls_by_L[L]
            if n_cols == 0:
                continue
            pair_idx, w_half = _pack_stream(idx2, w2, L, n_cols)
            m[f"idx{L}"] = _wrap_idxs(pair_idx)
            m[f"w{L}"] = w_half.reshape(P, n_cols * 2)
            meta[L] = (chunk_o, idx2.shape[0])
        in_maps.append(m)
        metas.append(meta)
    return in_maps, metas, n_calls_by_L


def _assemble(results, metas, n_calls_by_L):
    """Decode device outputs and accumulate per-chunk sums into (B, O)."""
    out = np.zeros((O, B), dtype=np.float32)
    for c in range(N_CORES):
        o0 = c * O_PER_CORE
        for L in CHUNK_SIZES:
            n_cols = n_calls_by_L[L]
            if n_cols == 0:
                continue
            chunk_o, n_chunks = metas[c][L]
            if n_chunks == 0:
                continue
            nb = P // L
            raw = results[c][f"out{L}"]           # (nb, n_cols*2*B)
            # [j, ((cc*2)+h)*B + b]: half-h partial sum of chunk cc*nb + j
            raw = raw.reshape(nb, n_cols, 2, B).sum(axis=2)
            sums = np.transpose(raw, (1, 0, 2)).reshape(n_cols * nb, B)
            np.add.at(out, chunk_o[:n_chunks] + o0, sums[:n_chunks])
    return np.ascontiguousarray(out.T)


def kernel(x, forward_weights, input_mapping, output_size):
    from concourse.bass_utils import run_bass_kernel_spmd

    x = np.asarray(x)
    forward_weights = np.asarray(forward_weights)
    input_mapping = np.asarray(input_mapping)
    assert int(output_size) == O

    in_maps, metas, n_calls_by_L = _prepare(x, forward_weights, input_mapping)

    key = tuple(sorted(n_calls_by_L.items()))
    if key not in _CACHE:
        _CACHE[key] = _build_program(n_calls_by_L)
    nc = _CACHE[key]

    res = run_bass_kernel_spmd(
        nc, in_maps, core_ids=list(range(N_CORES))
    )
    return _assemble(res.results, metas, n_calls_by_L)
